# revision 2
# baseline (speedup 1.0000x reference)
"""Multi-head causal+padded attention on 8 Trainium2 NeuronCores.

Sharding: core c handles batch b = c//2 and head-group g = c%2 (8 of 16 heads).
Each core computes its q/k/v projections (512 output dims) and attention for
its 8 heads over the full 2048-seq, producing out^T [512, 2048]; the host
transposes/concats into the full [4, 2048, 1024] output.

Device algorithm (per core):
  xT [1024,2048] resident in SBUF; qT/kT = W^T-slices @ xT (f32r matmuls,
  output layout [outdim, seq]); v in natural [seq, outdim] layout, bias-added,
  pad-masked, stored bf16 augmented with a 65th column = pad mask.
  Scores are computed transposed (sT[k,q] = k_h^T q_h) per 128-k-block, exp'd
  on the scalar engine (scale=1/8 folded in), causal-masked only on diagonal
  blocks, then att^T @ [v|pad] accumulates in PSUM giving both out^T[d,q] and
  the softmax denominator (row 64) in one accumulation chain. Normalization
  multiplies by the broadcast reciprocal (gpsimd partition_broadcast).
"""
import os
import sys

sys.path.insert(0, "/opt/trn_rl_repo")

import numpy as np

S = 2048
E = 1024
D = 64
H = 16          # total heads
HPC = 8         # heads per core
OC = HPC * D    # 512 output dims per core
EB = E // 128   # 8 contraction blocks
NSB = S // 128  # 16 seq blocks
NCH = S // 512  # 4 q-chunks
B = 4
NCORES = 8

_cache = {}


def _build_nc():
    from concourse import bacc
    import concourse.tile as tile
    import concourse.mybir as mybir

    F32 = mybir.dt.float32
    F32R = mybir.dt.float32r
    ATT_DT = {"bf16": mybir.dt.bfloat16, "f32r": F32R}[
        os.environ.get("MHA_ATT_DT", "bf16")
    ]
    AF = mybir.ActivationFunctionType

    nc = bacc.Bacc("TRN2", target_bir_lowering=False, debug=False,
                   num_devices=NCORES)
    xT = nc.dram_tensor("xT", [E, S], F32R, kind="ExternalInput").ap()
    wqT = nc.dram_tensor("wqT", [E, OC], F32R, kind="ExternalInput").ap()
    wkT = nc.dram_tensor("wkT", [E, OC], F32R, kind="ExternalInput").ap()
    wvT = nc.dram_tensor("wvT", [E, OC], F32R, kind="ExternalInput").ap()
    bq = nc.dram_tensor("bq", [OC], F32, kind="ExternalInput").ap()
    bk = nc.dram_tensor("bk", [OC], F32, kind="ExternalInput").ap()
    bv = nc.dram_tensor("bv", [OC], F32, kind="ExternalInput").ap()
    pad = nc.dram_tensor("pad", [S], F32, kind="ExternalInput").ap()
    outT = nc.dram_tensor("outT", [OC, S], F32, kind="ExternalOutput").ap()

    with tile.TileContext(nc) as tc:
        with tc.tile_pool(name="const", bufs=1) as cpool, \
             tc.tile_pool(name="big", bufs=1) as bigpool:

            # ---------------- constants ----------------
            pad_sb = cpool.tile([128, NSB], F32, tag="pad_f")
            nc.sync.dma_start(pad_sb[:], pad.rearrange("(b p) -> p b", p=128))
            pad_row = cpool.tile([1, S], F32, tag="padr_f")
            nc.sync.dma_start(pad_row[:], pad.rearrange("(a s) -> a s", a=1))

            bq_sb = cpool.tile([128, 4], F32, tag="bq")
            nc.sync.dma_start(bq_sb[:], bq.rearrange("(b p) -> p b", p=128))
            bk_sb = cpool.tile([128, 4], F32, tag="bk")
            nc.sync.dma_start(bk_sb[:], bk.rearrange("(b p) -> p b", p=128))
            bv_row = cpool.tile([1, OC], F32, tag="bv_row")
            nc.sync.dma_start(bv_row[:], bv.rearrange("(a c) -> a c", a=1))
            bv_tile = cpool.tile([128, OC], F32, tag="bv_tile")
            nc.gpsimd.partition_broadcast(bv_tile[:], bv_row[:])

            # tri[k, q] = 1 where k <= q else 0 (local 128x128 diagonal block)
            tri = cpool.tile([128, 128], ATT_DT, tag="tri")
            nc.gpsimd.memset(tri[:], 1.0)
            nc.gpsimd.affine_select(
                out=tri[:], in_=tri[:], compare_op=mybir.AluOpType.is_ge,
                fill=0.0, base=0, pattern=[[1, 128]], channel_multiplier=-1)

            qT_sb = bigpool.tile([128, 4 * S], F32R, tag="qT")
            kT_sb = bigpool.tile([128, 4 * S], F32R, tag="kT")
            v_aug = bigpool.tile([128, NSB * HPC * 65], ATT_DT, tag="v_aug")
            v_r = v_aug[:].rearrange("p (b h c) -> p b h c", b=NSB, h=HPC)

            # ======== phase 1: projections ========
            with tc.tile_pool(name="xw", bufs=3) as xw, \
                 tc.tile_pool(name="xp", bufs=1) as xp, \
                 tc.tile_pool(name="psP", bufs=4, space="PSUM") as psP:

                x_sb = xp.tile([128, EB * S], F32R, tag="x_sb")
                for eb in range(EB):
                    nc.sync.dma_start(x_sb[:, eb * S:(eb + 1) * S],
                                      xT[eb * 128:(eb + 1) * 128, :])

                # init the 65th (pad) columns of v_aug once
                nc.gpsimd.memset(v_r[:, :, :, 64], 1.0)

                def load_w(wdram):
                    halves = []
                    for half in range(2):
                        w_sb = xw.tile([128, 4 * OC], F32R, tag="w",
                                       name=f"w_{half}")
                        for i in range(4):
                            eb = half * 4 + i
                            nc.sync.dma_start(
                                w_sb[:, i * OC:(i + 1) * OC],
                                wdram[eb * 128:(eb + 1) * 128, :])
                        halves.append(w_sb)
                    return halves

                # q/k projections (out layout [o, s])
                for wdram, bias_sb, dst in ((wqT, bq_sb, qT_sb),
                                            (wkT, bk_sb, kT_sb)):
                    wh = load_w(wdram)
                    for ob in range(4):
                        for scn in range(4):
                            ps = psP.tile([128, 512], F32, tag="ps_proj")
                            for eb in range(EB):
                                w_sb = wh[eb // 4]
                                i = eb % 4
                                nc.tensor.matmul(
                                    ps[:],
                                    w_sb[:, i * OC + ob * 128:
                                         i * OC + (ob + 1) * 128],
                                    x_sb[:, eb * S + scn * 512:
                                         eb * S + (scn + 1) * 512],
                                    start=(eb == 0), stop=(eb == EB - 1))
                            nc.vector.tensor_scalar_add(
                                dst[:, ob * S + scn * 512:
                                    ob * S + (scn + 1) * 512],
                                ps[:], bias_sb[:, ob:ob + 1])

                # v projection (natural [s, o] layout, bias+pad, bf16)
                wh = load_w(wvT)
                for sb in range(NSB):
                    ps = psP.tile([128, 512], F32, tag="ps_proj")
                    for eb in range(EB):
                        w_sb = wh[eb // 4]
                        i = eb % 4
                        nc.tensor.matmul(
                            ps[:],
                            x_sb[:, eb * S + sb * 128:eb * S + (sb + 1) * 128],
                            w_sb[:, i * OC:(i + 1) * OC],
                            start=(eb == 0), stop=(eb == EB - 1))
                    nc.vector.tensor_add(
                        v_r[:, sb, :, 0:64],
                        ps[:].rearrange("p (h c) -> p h c", h=HPC),
                        bv_tile[:].rearrange("p (h c) -> p h c", h=HPC))
                    nc.vector.tensor_scalar_mul(
                        v_aug[:, sb * HPC * 65:(sb + 1) * HPC * 65],
                        v_aug[:, sb * HPC * 65:(sb + 1) * HPC * 65],
                        pad_sb[:, sb:sb + 1])

            stage = os.environ.get("MHA_STAGE", "full")
            if stage == "proj":
                with tc.tile_pool(name="dbg", bufs=2) as dbg:
                    nc.sync.dma_start(outT[0:128, :], qT_sb[:, 0:S])
                    nc.sync.dma_start(outT[128:256, :], kT_sb[:, 0:S])
                    vdump = dbg.tile([128, 512], F32, tag="vd")
                    nc.vector.tensor_copy(vdump[:], v_aug[:, 0:512])
                    nc.sync.dma_start(outT[256:384, 0:512], vdump[:])
                nc.compile()
                return nc

            # ======== phase 2: attention ========
            with tc.tile_pool(name="attp", bufs=6) as attp, \
                 tc.tile_pool(name="work", bufs=4) as work, \
                 tc.tile_pool(name="outp", bufs=3) as outp, \
                 tc.tile_pool(name="psS", bufs=4, space="PSUM") as psS, \
                 tc.tile_pool(name="psAv", bufs=2, space="PSUM") as psAv:

                for scn in range(NCH):
                    q0 = scn * 512
                    nkb = 4 * scn + 4
                    for hp in range(4):
                        heads = (2 * hp, 2 * hp + 1)
                        if stage != "noav":
                            avs = [psAv.tile([65, 512], F32, tag=f"ps_av{i}",
                                             name=f"ps_av{i}")
                                   for i in range(2)]
                        for kb in range(nkb):
                            lstart = max(0, kb * 128 - q0)
                            w = 512 - lstart
                            for i, h in enumerate(heads):
                                ob = h // 2
                                po = (h % 2) * 64
                                ssb = psS.tile([128, 512], F32, tag="ps_s")
                                nc.tensor.matmul(
                                    ssb[:, 0:w],
                                    kT_sb[po:po + 64,
                                          ob * S + kb * 128:
                                          ob * S + (kb + 1) * 128],
                                    qT_sb[po:po + 64,
                                          ob * S + q0 + lstart:ob * S + q0 + 512],
                                    start=True, stop=True)
                                att_t = attp.tile([128, 512], ATT_DT, tag="att")
                                nc.scalar.activation(att_t[:, 0:w], ssb[:, 0:w],
                                                     AF.Exp, scale=0.125)
                                if kb >= 4 * scn:
                                    nc.vector.tensor_mul(att_t[:, 0:128],
                                                         att_t[:, 0:128],
                                                         tri[:])
                                if stage != "noav":
                                    nc.tensor.matmul(
                                        avs[i][:, lstart:512],
                                        v_r[:, kb, h, :],
                                        att_t[:, 0:w],
                                        start=(kb == 0), stop=(kb == nkb - 1))
                                elif kb == nkb - 1:
                                    o_sb = outp.tile([64, 512], F32, tag="osb",
                                                     name="o_sb")
                                    nc.vector.tensor_copy(o_sb[:],
                                                          att_t[0:64, :])
                                    nc.sync.dma_start(
                                        outT[h * 64:(h + 1) * 64, q0:q0 + 512],
                                        o_sb[:])
                        if stage == "noav":
                            continue
                        for i, h in enumerate(heads):
                            r0 = work.tile([1, 512], F32, tag="rt", name="r0")
                            nc.vector.tensor_scalar_add(r0[:], avs[i][64:65, :],
                                                        1e-30)
                            r1 = work.tile([1, 512], F32, tag="rt", name="r1")
                            nc.vector.reciprocal(r1[:], r0[:])
                            r2 = work.tile([1, 512], F32, tag="rt", name="r2")
                            nc.vector.tensor_mul(r2[:], r1[:],
                                                 pad_row[:, q0:q0 + 512])
                            o_sb = outp.tile([64, 512], F32, tag="osb",
                                             name="o_sb")
                            if stage == "nobc":
                                nc.vector.tensor_copy(o_sb[:], avs[i][0:64, :])
                            else:
                                bc = work.tile([64, 512], F32, tag="bc",
                                               name="bc")
                                nc.gpsimd.partition_broadcast(bc[:], r2[:])
                                nc.vector.tensor_mul(o_sb[:], avs[i][0:64, :],
                                                     bc[:])
                            nc.sync.dma_start(
                                outT[h * 64:(h + 1) * 64, q0:q0 + 512],
                                o_sb[:])
    nc.compile()
    return nc


def get_nc():
    key = (os.environ.get("MHA_ATT_DT", "bf16"),
           os.environ.get("MHA_STAGE", "full"))
    if key not in _cache:
        _cache[key] = _build_nc()
    return _cache[key]


def make_in_maps(input_x, pad_mask, Wq, bq, Wk, bk, Wv, bv):
    input_x = np.asarray(input_x, dtype=np.float32)
    pad_f = np.asarray(pad_mask).astype(np.float32)
    Wq = np.asarray(Wq, dtype=np.float32)
    Wk = np.asarray(Wk, dtype=np.float32)
    Wv = np.asarray(Wv, dtype=np.float32)
    bq = np.asarray(bq, dtype=np.float32)
    bk = np.asarray(bk, dtype=np.float32)
    bv = np.asarray(bv, dtype=np.float32)

    xTs = [np.ascontiguousarray(input_x[b].T) for b in range(B)]
    wslices = {}
    for g in range(2):
        sl = slice(g * OC, (g + 1) * OC)
        wslices[g] = (np.ascontiguousarray(Wq[sl].T),
                      np.ascontiguousarray(Wk[sl].T),
                      np.ascontiguousarray(Wv[sl].T),
                      np.ascontiguousarray(bq[sl]),
                      np.ascontiguousarray(bk[sl]),
                      np.ascontiguousarray(bv[sl]))
    in_maps = []
    for c in range(NCORES):
        b, g = c // 2, c % 2
        wq_t, wk_t, wv_t, bq_s, bk_s, bv_s = wslices[g]
        in_maps.append({
            "xT": xTs[b], "wqT": wq_t, "wkT": wk_t, "wvT": wv_t,
            "bq": bq_s, "bk": bk_s, "bv": bv_s,
            "pad": np.ascontiguousarray(pad_f[b]),
        })
    return in_maps


def assemble(results):
    out = np.empty((B, S, E), dtype=np.float32)
    for c in range(NCORES):
        b, g = c // 2, c % 2
        out[b, :, g * OC:(g + 1) * OC] = results[c]["outT"].T
    return out


LAST_RESULT = None


def kernel(input_x, pad_mask, Wq, bq, Wk, bk, Wv, bv):
    from concourse.bass_utils import run_bass_kernel_spmd
    global LAST_RESULT
    nc = get_nc()
    in_maps = make_in_maps(input_x, pad_mask, Wq, bq, Wk, bk, Wv, bv)
    res = run_bass_kernel_spmd(nc, in_maps, core_ids=list(range(NCORES)))
    LAST_RESULT = res
    if res.exec_time_ns is not None:
        print(f"HW exec time: {res.exec_time_ns} ns")
    return assemble(res.results)



# revision 3
# speedup vs baseline: 3.0177x; 3.0177x over previous
"""Multi-head causal+padded attention on 8 Trainium2 NeuronCores.

Core c handles batch b = c//2 and head-group g = c%2 (8 of 16 heads).

Pad compaction: the reference masks out padded keys/queries entirely
(padded query rows output 0). Attention over the pad-compacted sequence is
exactly equivalent, so the host gathers the ~1024 unpadded rows per batch,
zero-pads to a fixed 1152 capacity, and the device runs a causal MHA on
[1152] — ~4x less attention work and ~2x less projection work. Outputs are
scattered back with zeros in padded rows.

Device (per core, all-bf16 datapath, fp32 PSUM):
  qT/kT = W^T-slices @ xT in [out, seq] layout; v in natural [seq, out]
  layout augmented with a ones column (row-sum accumulator -> softmax
  denominator rides along the att@v matmul chain).
  Scores are computed transposed per 128-k-block (head pairs packed into
  PE row-groups 0-1/2-3 so the two matmuls run concurrently), copied
  PSUM->SBUF by the DVE, exp'd in ONE big scalar-engine activation per
  (head, chunk) (amortizes the ~352-cycle ACT instruction overhead),
  tri-masked on diagonal blocks, then att^T-chained into [65, 384] PSUM.
  Unnormalized out + denominator go to HBM; the host divides.
  Projection chains are interleaved into the attention emission so the PE
  never idles long enough for HAM to re-throttle the clock.
"""
import os
import sys

sys.path.insert(0, "/opt/trn_rl_repo")

import numpy as np

E = 1024
D = 64
HPC = 8         # heads per core
OC = HPC * D    # 512 output dims per core
EB = E // 128   # 8 contraction blocks
B = 4
NCORES = 8
CH = 384        # q-chunk width
CAP0 = 1152     # default compacted seq capacity (multiple of 384)

_cache = {}


def _build_nc(seqc):
    from concourse import bacc
    import concourse.tile as tile
    import concourse.mybir as mybir

    assert seqc % CH == 0
    NCH = seqc // CH          # q-chunks (3 at cap 1152)
    NB = seqc // 128          # 128-k-blocks (9)
    F32 = mybir.dt.float32
    BF16 = mybir.dt.bfloat16
    AF = mybir.ActivationFunctionType

    nc = bacc.Bacc("TRN2", target_bir_lowering=False, debug=False,
                   num_devices=NCORES)
    xT = nc.dram_tensor("xT", [E, seqc], BF16, kind="ExternalInput").ap()
    wqT = nc.dram_tensor("wqT", [E, OC], BF16, kind="ExternalInput").ap()
    wkT = nc.dram_tensor("wkT", [E, OC], BF16, kind="ExternalInput").ap()
    wvT = nc.dram_tensor("wvT", [E, OC], BF16, kind="ExternalInput").ap()
    bq = nc.dram_tensor("bq", [OC], F32, kind="ExternalInput").ap()
    bk = nc.dram_tensor("bk", [OC], F32, kind="ExternalInput").ap()
    bv = nc.dram_tensor("bv", [OC], F32, kind="ExternalInput").ap()
    # unnormalized out (64 rows) + denominator (row 64) per head
    outT = nc.dram_tensor("outT", [HPC * 65, seqc], F32,
                          kind="ExternalOutput").ap()

    with tile.TileContext(nc) as tc:
        with tc.tile_pool(name="const", bufs=1) as cpool, \
             tc.tile_pool(name="big", bufs=1) as bigpool, \
             tc.tile_pool(name="psP", bufs=2, space="PSUM") as psP, \
             tc.tile_pool(name="psS", bufs=3, space="PSUM") as psS, \
             tc.tile_pool(name="psAv", bufs=3, space="PSUM") as psAv, \
             tc.tile_pool(name="sco", bufs=4) as sco_pool, \
             tc.tile_pool(name="att", bufs=4) as att_pool, \
             tc.tile_pool(name="outp", bufs=4) as out_pool:

            # ---------------- constants ----------------
            bq_sb = cpool.tile([128, 4], F32, tag="bq")
            nc.sync.dma_start(bq_sb[:], bq.rearrange("(b p) -> p b", p=128))
            bk_sb = cpool.tile([128, 4], F32, tag="bk")
            nc.sync.dma_start(bk_sb[:], bk.rearrange("(b p) -> p b", p=128))
            bv_row = cpool.tile([1, OC], F32, tag="bv_row")
            nc.sync.dma_start(bv_row[:], bv.rearrange("(a c) -> a c", a=1))
            bv_tile = cpool.tile([128, OC], F32, tag="bv_tile")
            nc.gpsimd.partition_broadcast(bv_tile[:], bv_row[:])

            # tri[k, q] = 1 where k <= q else 0 (diagonal 128x128 block)
            tri = cpool.tile([128, 128], BF16, tag="tri")
            nc.gpsimd.memset(tri[:], 1.0)
            nc.gpsimd.affine_select(
                out=tri[:], in_=tri[:], compare_op=mybir.AluOpType.is_ge,
                fill=0.0, base=0, pattern=[[1, 128]], channel_multiplier=-1)

            # ---------------- persistent SBUF ----------------
            x_sb = bigpool.tile([128, EB * seqc], BF16, tag="x_sb")
            wq_sb = bigpool.tile([128, EB * OC], BF16, tag="wq_sb")
            wk_sb = bigpool.tile([128, EB * OC], BF16, tag="wk_sb")
            wv_sb = bigpool.tile([128, EB * OC], BF16, tag="wv_sb")
            qT_sb = bigpool.tile([128, 4 * seqc], BF16, tag="qT")
            kT_sb = bigpool.tile([128, 4 * seqc], BF16, tag="kT")
            v_aug = bigpool.tile([128, NB * HPC * 65], BF16, tag="v_aug")
            v_r = v_aug[:].rearrange("p (b h c) -> p b h c", b=NB, h=HPC)

            for eb in range(EB):
                nc.sync.dma_start(wk_sb[:, eb * OC:(eb + 1) * OC],
                                  wkT[eb * 128:(eb + 1) * 128, :])
            for eb in range(EB):
                nc.sync.dma_start(x_sb[:, eb * seqc:(eb + 1) * seqc],
                                  xT[eb * 128:(eb + 1) * 128, :])
            for eb in range(EB):
                nc.sync.dma_start(wq_sb[:, eb * OC:(eb + 1) * OC],
                                  wqT[eb * 128:(eb + 1) * 128, :])
            for eb in range(EB):
                nc.sync.dma_start(wv_sb[:, eb * OC:(eb + 1) * OC],
                                  wvT[eb * 128:(eb + 1) * 128, :])

            nc.gpsimd.memset(v_r[:, :, :, 64], 1.0)

            # ---------------- projection chain emitters ----------------
            def emit_qk_chains(w_sb, bias_sb, dst, obs, ch):
                """Chains for out-dim blocks `obs`, q-col chunk `ch`;
                eb-major across the group so work starts on partial x."""
                pss = [psP.tile([128, 512], F32, tag="ps_proj",
                                name=f"pp{ob}") for ob in obs]
                for eb in range(EB):
                    for ps, ob in zip(pss, obs):
                        nc.tensor.matmul(
                            ps[:, 0:CH],
                            w_sb[:, eb * OC + ob * 128:eb * OC + (ob + 1) * 128],
                            x_sb[:, eb * seqc + ch * CH:eb * seqc + (ch + 1) * CH],
                            start=(eb == 0), stop=(eb == EB - 1))
                for ps, ob in zip(pss, obs):
                    nc.vector.tensor_scalar_add(
                        dst[:, ob * seqc + ch * CH:ob * seqc + (ch + 1) * CH],
                        ps[:, 0:CH], bias_sb[:, ob:ob + 1])

            def emit_v_chain(sb):
                ps = psP.tile([128, 512], F32, tag="ps_proj")
                for eb in range(EB):
                    nc.tensor.matmul(
                        ps[:],
                        x_sb[:, eb * seqc + sb * 128:eb * seqc + (sb + 1) * 128],
                        wv_sb[:, eb * OC:(eb + 1) * OC],
                        start=(eb == 0), stop=(eb == EB - 1))
                nc.vector.tensor_add(
                    v_r[:, sb, :, 0:64],
                    ps[:].rearrange("p (h c) -> p h c", h=HPC),
                    bv_tile[:].rearrange("p (h c) -> p h c", h=HPC))

            # ---------------- attention emitters ----------------
            def widths(scn):
                """[(kb, off, w, lstart)] for chunk scn, packed offsets."""
                q0 = scn * CH
                out, off = [], 0
                for kb in range(3 * scn + 3):
                    lstart = max(0, kb * 128 - q0)
                    w = CH - lstart
                    out.append((kb, off, w, lstart))
                    off += w
                return out

            state = {}

            def emit_scores(scn, hp):
                wl = widths(scn)
                sw = sum(w for _, _, w, _ in wl)
                q0 = scn * CH
                scos = []
                for i in range(2):
                    scos.append(sco_pool.tile([128, sw], BF16, tag="sco",
                                              name=f"sco{i}"))
                for kb, off, w, lstart in wl:
                    for i in range(2):
                        h = 2 * hp + i
                        ob, po = h // 2, (h % 2) * 64
                        ssb = psS.tile([128, 512], F32, tag="ps_s")
                        nc.tensor.matmul(
                            ssb[:, 0:w],
                            kT_sb[po:po + 64,
                                  ob * seqc + kb * 128:ob * seqc + (kb + 1) * 128],
                            qT_sb[po:po + 64,
                                  ob * seqc + q0 + lstart:ob * seqc + q0 + CH],
                            start=True, stop=True)
                        nc.vector.tensor_copy(scos[i][:, off:off + w],
                                              ssb[:, 0:w])
                state[(scn, hp)] = scos

            def emit_exp_av_out(scn, hp):
                wl = widths(scn)
                sw = sum(w for _, _, w, _ in wl)
                q0 = scn * CH
                scos = state.pop((scn, hp))
                atts = []
                for i in range(2):
                    att = att_pool.tile([128, sw], BF16, tag="att",
                                        name=f"att{i}")
                    nc.scalar.activation(att[:], scos[i][:], AF.Exp,
                                         scale=0.125)
                    for kb, off, w, lstart in wl:
                        if kb >= 3 * scn:  # diagonal block: causal tri mask
                            nc.vector.tensor_mul(att[:, off:off + 128],
                                                 att[:, off:off + 128], tri[:])
                    atts.append(att)
                avs = [psAv.tile([65, 512], F32, tag="ps_av", name=f"av{i}")
                       for i in range(2)]
                nkb = 3 * scn + 3
                for kb, off, w, lstart in wl:
                    for i in range(2):
                        h = 2 * hp + i
                        nc.tensor.matmul(
                            avs[i][:, lstart:CH],
                            v_r[:, kb, h, :],
                            atts[i][:, off:off + w],
                            start=(kb == 0), stop=(kb == nkb - 1))
                for i in range(2):
                    h = 2 * hp + i
                    o_sb = out_pool.tile([65, CH], F32, tag="osb",
                                         name="o_sb")
                    nc.vector.tensor_copy(o_sb[:], avs[i][:, 0:CH])
                    nc.sync.dma_start(
                        outT[h * 65:(h + 1) * 65, q0:q0 + CH], o_sb[:])

            # ---------------- schedule ----------------
            # Upfront: k chunk 0 (eb-major pairs), v blocks 0-2, q chunk 0.
            emit_qk_chains(wk_sb, bk_sb, kT_sb, (0, 1), 0)
            emit_qk_chains(wk_sb, bk_sb, kT_sb, (2, 3), 0)
            for sb in range(3):
                emit_v_chain(sb)
            emit_qk_chains(wq_sb, bq_sb, qT_sb, (0, 1), 0)
            emit_qk_chains(wq_sb, bq_sb, qT_sb, (2, 3), 0)

            # Remaining projection work interleaved into attention slots.
            # slot (scn, hp) -> list of emitters; chunk c chains must land
            # before S(c, 0) is emitted (one slot of lookahead).
            def mk_sched(NCH):
                sched = {}
                if NCH != 3:
                    return sched  # fallback handled below (all upfront)
                sched[(0, 0)] = [lambda: emit_qk_chains(wk_sb, bk_sb, kT_sb,
                                                        (0, 1), 1)]
                sched[(0, 1)] = [lambda: emit_qk_chains(wk_sb, bk_sb, kT_sb,
                                                        (2, 3), 1),
                                 lambda: emit_v_chain(3)]
                sched[(0, 2)] = [lambda: emit_qk_chains(wq_sb, bq_sb, qT_sb,
                                                        (0, 1), 1),
                                 lambda: emit_v_chain(4)]
                sched[(0, 3)] = [lambda: emit_qk_chains(wq_sb, bq_sb, qT_sb,
                                                        (2, 3), 1),
                                 lambda: emit_v_chain(5)]
                sched[(1, 0)] = [lambda: emit_qk_chains(wk_sb, bk_sb, kT_sb,
                                                        (0, 1), 2)]
                sched[(1, 1)] = [lambda: emit_qk_chains(wk_sb, bk_sb, kT_sb,
                                                        (2, 3), 2),
                                 lambda: emit_v_chain(6)]
                sched[(1, 2)] = [lambda: emit_qk_chains(wq_sb, bq_sb, qT_sb,
                                                        (0, 1), 2),
                                 lambda: emit_v_chain(7)]
                sched[(1, 3)] = [lambda: emit_qk_chains(wq_sb, bq_sb, qT_sb,
                                                        (2, 3), 2),
                                 lambda: emit_v_chain(8)]
                return sched

            sched = mk_sched(NCH)
            if not sched and NCH != 3:
                for ch in range(1, NCH):
                    emit_qk_chains(wk_sb, bk_sb, kT_sb, (0, 1), ch)
                    emit_qk_chains(wk_sb, bk_sb, kT_sb, (2, 3), ch)
                    emit_qk_chains(wq_sb, bq_sb, qT_sb, (0, 1), ch)
                    emit_qk_chains(wq_sb, bq_sb, qT_sb, (2, 3), ch)
                for sb in range(3, NB):
                    emit_v_chain(sb)

            pairs = [(scn, hp) for scn in range(NCH) for hp in range(4)]
            emit_scores(*pairs[0])
            for j, p in enumerate(pairs):
                for fn in sched.get(p, ()):
                    fn()
                if j + 1 < len(pairs):
                    emit_scores(*pairs[j + 1])
                emit_exp_av_out(*p)

    nc.compile()
    return nc


def get_nc(seqc=CAP0):
    if seqc not in _cache:
        _cache[seqc] = _build_nc(seqc)
    return _cache[seqc]


def _prep(input_x, pad_mask, Wq, bq, Wk, bk, Wv, bv):
    import ml_dtypes
    bf16 = ml_dtypes.bfloat16
    input_x = np.asarray(input_x, dtype=np.float32)
    pad = np.asarray(pad_mask)
    Ws = [np.asarray(w, dtype=np.float32) for w in (Wq, Wk, Wv)]
    bs = [np.ascontiguousarray(np.asarray(v, dtype=np.float32))
          for v in (bq, bk, bv)]

    idxs = [np.flatnonzero(pad[b]) for b in range(B)]
    sbs = [len(ix) for ix in idxs]
    cap = max(CAP0, -(-max(sbs) // CH) * CH)

    xTs = []
    for b in range(B):
        xc = np.zeros((cap, E), np.float32)
        xc[:sbs[b]] = input_x[b][idxs[b]]
        xTs.append(np.ascontiguousarray(xc.T).astype(bf16))

    wslices = {}
    for g in range(2):
        sl = slice(g * OC, (g + 1) * OC)
        wslices[g] = tuple(
            np.ascontiguousarray(W[sl].T).astype(bf16) for W in Ws
        ) + tuple(np.ascontiguousarray(v[sl]) for v in bs)

    in_maps = []
    for c in range(NCORES):
        b, g = c // 2, c % 2
        wq_t, wk_t, wv_t, bq_s, bk_s, bv_s = wslices[g]
        in_maps.append({
            "xT": xTs[b], "wqT": wq_t, "wkT": wk_t, "wvT": wv_t,
            "bq": bq_s, "bk": bk_s, "bv": bv_s,
        })
    return in_maps, idxs, sbs, cap


def _assemble(results, idxs, sbs, S):
    out = np.zeros((B, S, E), dtype=np.float32)
    for c in range(NCORES):
        b, g = c // 2, c % 2
        arr = results[c]["outT"]  # [520, cap] f32
        nb = sbs[b]
        for h in range(HPC):
            blk = arr[h * 65:(h + 1) * 65, :nb]
            o = blk[:64] / blk[64:65]
            out[b, idxs[b], g * OC + h * 64:g * OC + (h + 1) * 64] = o.T
    return out


LAST_RESULT = None


def kernel(input_x, pad_mask, Wq, bq, Wk, bk, Wv, bv):
    from concourse.bass_utils import run_bass_kernel_spmd
    global LAST_RESULT
    S = np.asarray(input_x).shape[1]
    in_maps, idxs, sbs, cap = _prep(input_x, pad_mask, Wq, bq, Wk, bk, Wv, bv)
    nc = get_nc(cap)
    res = run_bass_kernel_spmd(nc, in_maps, core_ids=list(range(NCORES)))
    LAST_RESULT = res
    if res.exec_time_ns is not None:
        print(f"HW exec time: {res.exec_time_ns} ns")
    return _assemble(res.results, idxs, sbs, S)


# revision 4
# speedup vs baseline: 3.0582x; 1.0134x over previous
"""Multi-head causal+padded attention on 8 Trainium2 NeuronCores.

Core c handles batch b = c//2 and head-group g = c%2 (8 of 16 heads).

Pad compaction: the reference masks out padded keys/queries entirely
(padded query rows output 0). Attention over the pad-compacted sequence is
exactly equivalent, so the host gathers the ~1024 unpadded rows per batch,
zero-pads to a fixed 1152 capacity, and the device runs a causal MHA on
[1152]. Outputs are scattered back with zeros in padded rows.

Device (per core, all-bf16 datapath, fp32 PSUM):
  qT/kT = W^T-slices @ xT in [out, seq] layout; v in natural [seq, out]
  layout augmented with a ones column (softmax denominator rides along the
  att@v accumulation chain). Scores transposed per 128-k-block, head pairs
  packed into PE row-groups 0-1/2-3 (concurrent matmuls), DVE-copied
  PSUM->SBUF, one batched exp per head-pair-chunk on the scalar engine,
  tri-masked on GpSimd, then att^T-chained into [65, 384] PSUM; ScalarE
  evacuates unnormalized out+denominator; the host divides.

  Emission is software-pipelined with stage offsets so no engine queue
  blocks another: slot j emits [proj chains | exp+mask(j+1) | scores(j+2)
  interleaved with av(j) | out(j)]. Projection chains are spread across
  attention slots (eb-major groups at startup to overlap the x DMA) so the
  PE never idles long enough for HAM to re-throttle the clock.
"""
import os
import sys

sys.path.insert(0, "/opt/trn_rl_repo")

import numpy as np

E = 1024
D = 64
HPC = 8         # heads per core
OC = HPC * D    # 512 output dims per core
EB = E // 128   # 8 contraction blocks
B = 4
NCORES = 8
CH = 384        # q-chunk width
CAP0 = 1152     # default compacted seq capacity (multiple of 384)

_cache = {}


def _build_nc(seqc):
    from concourse import bacc
    import concourse.tile as tile
    import concourse.mybir as mybir

    assert seqc % CH == 0
    NCH = seqc // CH          # q-chunks (3 at cap 1152)
    NB = seqc // 128          # 128-k-blocks (9)
    F32 = mybir.dt.float32
    BF16 = mybir.dt.bfloat16
    AF = mybir.ActivationFunctionType

    nc = bacc.Bacc("TRN2", target_bir_lowering=False, debug=False,
                   num_devices=NCORES)
    xT = nc.dram_tensor("xT", [E, seqc], BF16, kind="ExternalInput").ap()
    wqT = nc.dram_tensor("wqT", [E, OC], BF16, kind="ExternalInput").ap()
    wkT = nc.dram_tensor("wkT", [E, OC], BF16, kind="ExternalInput").ap()
    wvT = nc.dram_tensor("wvT", [E, OC], BF16, kind="ExternalInput").ap()
    bq = nc.dram_tensor("bq", [OC], F32, kind="ExternalInput").ap()
    bk = nc.dram_tensor("bk", [OC], F32, kind="ExternalInput").ap()
    bv = nc.dram_tensor("bv", [OC], F32, kind="ExternalInput").ap()
    # unnormalized out (64 rows) + denominator (row 64) per head
    outT = nc.dram_tensor("outT", [HPC * 65, seqc], F32,
                          kind="ExternalOutput").ap()

    with tile.TileContext(nc) as tc:
        with tc.tile_pool(name="const", bufs=1) as cpool, \
             tc.tile_pool(name="big", bufs=1) as bigpool, \
             tc.tile_pool(name="psP", bufs=3, space="PSUM") as psP, \
             tc.tile_pool(name="psS", bufs=3, space="PSUM") as psS, \
             tc.tile_pool(name="psAv", bufs=2, space="PSUM") as psAv, \
             tc.tile_pool(name="sco", bufs=3) as sco_pool, \
             tc.tile_pool(name="att", bufs=3) as att_pool, \
             tc.tile_pool(name="outp", bufs=4) as out_pool:

            # ---------------- constants ----------------
            bq_sb = cpool.tile([128, 4], F32, tag="bq")
            nc.sync.dma_start(bq_sb[:], bq.rearrange("(b p) -> p b", p=128))
            bk_sb = cpool.tile([128, 4], F32, tag="bk")
            nc.sync.dma_start(bk_sb[:], bk.rearrange("(b p) -> p b", p=128))
            bv_row = cpool.tile([1, OC], F32, tag="bv_row")
            nc.sync.dma_start(bv_row[:], bv.rearrange("(a c) -> a c", a=1))
            bv_tile = cpool.tile([128, OC], F32, tag="bv_tile")
            nc.gpsimd.partition_broadcast(bv_tile[:], bv_row[:])

            # tri[k, q] = 1 where k <= q else 0 (diagonal 128x128 block)
            tri = cpool.tile([128, 128], BF16, tag="tri")
            nc.gpsimd.memset(tri[:], 1.0)
            nc.gpsimd.affine_select(
                out=tri[:], in_=tri[:], compare_op=mybir.AluOpType.is_ge,
                fill=0.0, base=0, pattern=[[1, 128]], channel_multiplier=-1)

            # ---------------- persistent SBUF ----------------
            x_sb = bigpool.tile([128, EB * seqc], BF16, tag="x_sb")
            wq_sb = bigpool.tile([128, EB * OC], BF16, tag="wq_sb")
            wk_sb = bigpool.tile([128, EB * OC], BF16, tag="wk_sb")
            wv_sb = bigpool.tile([128, EB * OC], BF16, tag="wv_sb")
            qT_sb = bigpool.tile([128, 4 * seqc], BF16, tag="qT")
            kT_sb = bigpool.tile([128, 4 * seqc], BF16, tag="kT")
            v_aug = bigpool.tile([128, NB * HPC * 65], BF16, tag="v_aug")
            v_r = v_aug[:].rearrange("p (b h c) -> p b h c", b=NB, h=HPC)

            # wk/x interleaved so the first eb-major chains start early
            for eb in range(EB):
                nc.sync.dma_start(wk_sb[:, eb * OC:(eb + 1) * OC],
                                  wkT[eb * 128:(eb + 1) * 128, :])
                nc.sync.dma_start(x_sb[:, eb * seqc:(eb + 1) * seqc],
                                  xT[eb * 128:(eb + 1) * 128, :])
            for eb in range(EB):
                nc.sync.dma_start(wq_sb[:, eb * OC:(eb + 1) * OC],
                                  wqT[eb * 128:(eb + 1) * 128, :])
            for eb in range(EB):
                nc.sync.dma_start(wv_sb[:, eb * OC:(eb + 1) * OC],
                                  wvT[eb * 128:(eb + 1) * 128, :])

            nc.gpsimd.memset(v_r[:, :, :, 64], 1.0)

            # ---------------- projection chain groups ----------------
            # chain spec: ("k"|"q", ob, ch) weight-stationary, or ("v", sb)
            def emit_group(chains):
                pss = []
                for idx, c in enumerate(chains):
                    ps = psP.tile([128, 512], F32, tag="ps_proj",
                                  name=f"pp{idx}")
                    pss.append(ps)
                for eb in range(EB):
                    for c, ps in zip(chains, pss):
                        if c[0] == "v":
                            sb = c[1]
                            nc.tensor.matmul(
                                ps[:],
                                x_sb[:, eb * seqc + sb * 128:
                                     eb * seqc + (sb + 1) * 128],
                                wv_sb[:, eb * OC:(eb + 1) * OC],
                                start=(eb == 0), stop=(eb == EB - 1))
                        else:
                            _, ob, ch = c
                            w_sb = wk_sb if c[0] == "k" else wq_sb
                            nc.tensor.matmul(
                                ps[:, 0:CH],
                                w_sb[:, eb * OC + ob * 128:
                                     eb * OC + (ob + 1) * 128],
                                x_sb[:, eb * seqc + ch * CH:
                                     eb * seqc + (ch + 1) * CH],
                                start=(eb == 0), stop=(eb == EB - 1))
                for c, ps in zip(chains, pss):
                    if c[0] == "v":
                        sb = c[1]
                        nc.vector.tensor_add(
                            v_r[:, sb, :, 0:64],
                            ps[:].rearrange("p (h c) -> p h c", h=HPC),
                            bv_tile[:].rearrange("p (h c) -> p h c", h=HPC))
                    else:
                        _, ob, ch = c
                        dst = kT_sb if c[0] == "k" else qT_sb
                        bias_sb = bk_sb if c[0] == "k" else bq_sb
                        nc.vector.tensor_scalar_add(
                            dst[:, ob * seqc + ch * CH:
                                ob * seqc + (ch + 1) * CH],
                            ps[:, 0:CH], bias_sb[:, ob:ob + 1])

            # ---------------- attention emitters ----------------
            def widths(scn):
                """[(kb, off, w, lstart)] for chunk scn, packed offsets."""
                q0 = scn * CH
                out, off = [], 0
                for kb in range(3 * scn + 3):
                    lstart = max(0, kb * 128 - q0)
                    w = CH - lstart
                    out.append((kb, off, w, lstart))
                    off += w
                return out

            state = {}

            def emit_score_kb(p, sco, sw, item):
                scn, hp = p
                q0 = scn * CH
                kb, off, w, lstart = item
                for i in range(2):
                    h = 2 * hp + i
                    ob, po = h // 2, (h % 2) * 64
                    ssb = psS.tile([128, 512], F32, tag="ps_s")
                    nc.tensor.matmul(
                        ssb[:, 0:w],
                        kT_sb[po:po + 64,
                              ob * seqc + kb * 128:ob * seqc + (kb + 1) * 128],
                        qT_sb[po:po + 64,
                              ob * seqc + q0 + lstart:ob * seqc + q0 + CH],
                        start=True, stop=True)
                    nc.vector.tensor_copy(sco[:, i * sw + off:i * sw + off + w],
                                          ssb[:, 0:w])

            def emit_av_kb(p, att, sw, avs, item, nkb):
                scn, hp = p
                kb, off, w, lstart = item
                for i in range(2):
                    h = 2 * hp + i
                    nc.tensor.matmul(
                        avs[i][:, lstart:CH],
                        v_r[:, kb, h, :],
                        att[:, i * sw + off:i * sw + off + w],
                        start=(kb == 0), stop=(kb == nkb - 1))

            def emit_scores_plain(p):
                wl = widths(p[0])
                sw = sum(w for _, _, w, _ in wl)
                sco = sco_pool.tile([128, 2 * sw], BF16, tag="sco")
                for item in wl:
                    emit_score_kb(p, sco, sw, item)
                state[("sco", p)] = (sco, sw)

            def emit_exp_mask(p):
                scn, hp = p
                sco, sw = state.pop(("sco", p))
                att = att_pool.tile([128, 2 * sw], BF16, tag="att")
                nc.scalar.activation(att[:], sco[:], AF.Exp, scale=0.125)
                for kb, off, w, lstart in widths(scn):
                    if kb >= 3 * scn:  # diagonal block: causal tri mask
                        for i in range(2):
                            nc.gpsimd.tensor_mul(
                                att[:, i * sw + off:i * sw + off + 128],
                                att[:, i * sw + off:i * sw + off + 128],
                                tri[:])
                state[("att", p)] = (att, sw)

            def emit_av_scores(p_av, p_sco):
                """av MMs of p_av interleaved (PE-queue) with score MMs of
                p_sco so cast-paced score stalls are absorbed by av work."""
                av_items = widths(p_av[0]) if p_av else []
                nkb = len(av_items)
                if p_av:
                    att, sw_a = state.pop(("att", p_av))
                    avs = [psAv.tile([65, 512], F32, tag="ps_av",
                                     name=f"av{i}") for i in range(2)]
                    state[("avs", p_av)] = avs
                sco_items = widths(p_sco[0]) if p_sco else []
                if p_sco:
                    sw_s = sum(w for _, _, w, _ in sco_items)
                    sco = sco_pool.tile([128, 2 * sw_s], BF16, tag="sco")
                    state[("sco", p_sco)] = (sco, sw_s)
                for t in range(max(len(av_items), len(sco_items))):
                    if t < len(sco_items):
                        emit_score_kb(p_sco, sco, sw_s, sco_items[t])
                    if t < len(av_items):
                        emit_av_kb(p_av, att, sw_a, avs, av_items[t], nkb)

            def emit_out(p):
                scn, hp = p
                q0 = scn * CH
                avs = state.pop(("avs", p))
                for i in range(2):
                    h = 2 * hp + i
                    o_sb = out_pool.tile([65, CH], F32, tag="osb",
                                         name="o_sb")
                    nc.scalar.copy(o_sb[:], avs[i][:, 0:CH])
                    nc.sync.dma_start(
                        outT[h * 65:(h + 1) * 65, q0:q0 + CH], o_sb[:])

            # ---------------- schedule ----------------
            # Upfront eb-major groups overlap the x DMA; k/q chunk 0 and
            # v blocks 0-2 must precede the first attention slot.
            emit_group([("k", 0, 0), ("k", 1, 0), ("k", 2, 0)])
            emit_group([("k", 3, 0), ("q", 0, 0), ("q", 1, 0)])
            emit_group([("q", 2, 0), ("q", 3, 0), ("v", 0)])
            emit_group([("v", 1), ("v", 2)])

            # remaining chains spread over attention slots (need-by safe:
            # S(c,*) needs kT chunks<=c + qT chunk c two slots early; av(c,*)
            # needs v blocks < 3c+3 by its own slot)
            def G(*chains):
                return lambda: emit_group(list(chains))
            sched = {}
            if NCH == 3:
                sched = {
                    (0, 0): [G(("k", 0, 1), ("k", 1, 1))],
                    (0, 1): [G(("k", 2, 1), ("k", 3, 1), ("v", 3))],
                    (0, 2): [G(("q", 0, 1), ("q", 1, 1), ("v", 4))],
                    (0, 3): [G(("q", 2, 1), ("q", 3, 1), ("v", 5))],
                    (1, 0): [G(("k", 0, 2), ("k", 1, 2))],
                    (1, 1): [G(("k", 2, 2), ("k", 3, 2), ("v", 6))],
                    (1, 2): [G(("q", 0, 2), ("q", 1, 2), ("v", 7))],
                    (1, 3): [G(("q", 2, 2), ("q", 3, 2))],
                    (2, 0): [G(("v", 8))],
                }
            else:
                for ch in range(1, NCH):
                    for ob in range(0, 4, 2):
                        emit_group([("k", ob, ch), ("k", ob + 1, ch)])
                        emit_group([("q", ob, ch), ("q", ob + 1, ch)])
                for sb in range(3, NB):
                    emit_group([("v", sb)])

            pairs = [(scn, hp) for scn in range(NCH) for hp in range(4)]
            n = len(pairs)
            emit_scores_plain(pairs[0])
            emit_scores_plain(pairs[1])
            emit_exp_mask(pairs[0])
            for j, p in enumerate(pairs):
                for fn in sched.get(p, ()):
                    fn()
                if j + 1 < n:
                    emit_exp_mask(pairs[j + 1])
                emit_av_scores(p, pairs[j + 2] if j + 2 < n else None)
                emit_out(p)

    nc.compile()
    return nc


def get_nc(seqc=CAP0):
    if seqc not in _cache:
        _cache[seqc] = _build_nc(seqc)
    return _cache[seqc]


def _prep(input_x, pad_mask, Wq, bq, Wk, bk, Wv, bv):
    import ml_dtypes
    bf16 = ml_dtypes.bfloat16
    input_x = np.asarray(input_x, dtype=np.float32)
    pad = np.asarray(pad_mask)
    Ws = [np.asarray(w, dtype=np.float32) for w in (Wq, Wk, Wv)]
    bs = [np.ascontiguousarray(np.asarray(v, dtype=np.float32))
          for v in (bq, bk, bv)]

    idxs = [np.flatnonzero(pad[b]) for b in range(B)]
    sbs = [len(ix) for ix in idxs]
    cap = max(CAP0, -(-max(sbs) // CH) * CH)

    xTs = []
    for b in range(B):
        xc = np.zeros((cap, E), np.float32)
        xc[:sbs[b]] = input_x[b][idxs[b]]
        xTs.append(np.ascontiguousarray(xc.T).astype(bf16))

    wslices = {}
    for g in range(2):
        sl = slice(g * OC, (g + 1) * OC)
        wslices[g] = tuple(
            np.ascontiguousarray(W[sl].T).astype(bf16) for W in Ws
        ) + tuple(np.ascontiguousarray(v[sl]) for v in bs)

    in_maps = []
    for c in range(NCORES):
        b, g = c // 2, c % 2
        wq_t, wk_t, wv_t, bq_s, bk_s, bv_s = wslices[g]
        in_maps.append({
            "xT": xTs[b], "wqT": wq_t, "wkT": wk_t, "wvT": wv_t,
            "bq": bq_s, "bk": bk_s, "bv": bv_s,
        })
    return in_maps, idxs, sbs, cap


def _assemble(results, idxs, sbs, S):
    out = np.zeros((B, S, E), dtype=np.float32)
    for c in range(NCORES):
        b, g = c // 2, c % 2
        arr = results[c]["outT"]  # [520, cap] f32
        nb = sbs[b]
        for h in range(HPC):
            blk = arr[h * 65:(h + 1) * 65, :nb]
            o = blk[:64] / blk[64:65]
            out[b, idxs[b], g * OC + h * 64:g * OC + (h + 1) * 64] = o.T
    return out


LAST_RESULT = None


def kernel(input_x, pad_mask, Wq, bq, Wk, bk, Wv, bv):
    from concourse.bass_utils import run_bass_kernel_spmd
    global LAST_RESULT
    S = np.asarray(input_x).shape[1]
    in_maps, idxs, sbs, cap = _prep(input_x, pad_mask, Wq, bq, Wk, bk, Wv, bv)
    nc = get_nc(cap)
    res = run_bass_kernel_spmd(nc, in_maps, core_ids=list(range(NCORES)))
    LAST_RESULT = res
    if res.exec_time_ns is not None:
        print(f"HW exec time: {res.exec_time_ns} ns")
    return _assemble(res.results, idxs, sbs, S)


# revision 9
# speedup vs baseline: 4.0518x; 1.3249x over previous
"""Multi-head causal+padded attention on 8 Trainium2 NeuronCores.

Core c handles batch b = c//2 and head-group g = c%2 (8 of 16 heads).

Pad compaction: the reference masks out padded keys/queries entirely
(padded query rows output 0). Attention over the pad-compacted sequence is
exactly equivalent, so the host gathers the ~1024 unpadded rows per batch,
zero-pads to a fixed 1152 capacity, and the device runs a causal MHA on
[1152]. Outputs are scattered back with zeros in padded rows.

Device (per core, all-bf16 datapath, fp32 PSUM):
  qT/kT = W^T-slices @ xT in [out, seq] layout; v in natural [seq, out]
  layout augmented with a ones column (softmax denominator rides along the
  att@v accumulation chain). Scores transposed per 128-k-block, head pairs
  packed into PE row-groups 0-1/2-3 (concurrent matmuls), DVE-copied
  PSUM->SBUF, one batched exp per head-pair-chunk on the scalar engine,
  tri-masked on GpSimd, then att^T-chained into [65, 384] PSUM; ScalarE
  evacuates unnormalized out+denominator; the host divides.

  Emission is software-pipelined with stage offsets so no engine queue
  blocks another: slot j emits [proj chains | exp+mask(j+1) | scores(j+2)
  interleaved with av(j) | out(j)]. Projection chains are spread across
  attention slots (eb-major groups at startup to overlap the x DMA) so the
  PE never idles long enough for HAM to re-throttle the clock.
"""
import os
import sys

sys.path.insert(0, "/opt/trn_rl_repo")

import numpy as np

E = 1024
D = 64
HPC = 8         # heads per core
OC = HPC * D    # 512 output dims per core
EB = E // 128   # 8 contraction blocks
B = 4
NCORES = 8
CH = 384        # q-chunk width
CAP0 = 1152     # default compacted seq capacity (multiple of 384)

_cache = {}


def _build_nc(seqc):
    from concourse import bacc
    import concourse.tile as tile
    import concourse.mybir as mybir

    assert seqc % CH == 0
    NCH = seqc // CH          # q-chunks (3 at cap 1152)
    NB = seqc // 128          # 128-k-blocks (9)
    F32 = mybir.dt.float32
    BF16 = mybir.dt.bfloat16
    AF = mybir.ActivationFunctionType

    nc = bacc.Bacc("TRN2", target_bir_lowering=False, debug=False,
                   num_devices=NCORES)
    xT = nc.dram_tensor("xT", [E, seqc], BF16, kind="ExternalInput").ap()
    wqT = nc.dram_tensor("wqT", [E, OC], BF16, kind="ExternalInput").ap()
    wkT = nc.dram_tensor("wkT", [E, OC], BF16, kind="ExternalInput").ap()
    wvT = nc.dram_tensor("wvT", [E, OC], BF16, kind="ExternalInput").ap()
    bq = nc.dram_tensor("bq", [OC], F32, kind="ExternalInput").ap()
    bk = nc.dram_tensor("bk", [OC], F32, kind="ExternalInput").ap()
    bv = nc.dram_tensor("bv", [OC], F32, kind="ExternalInput").ap()
    # unnormalized out (64 rows) + denominator (row 64) per head
    outT = nc.dram_tensor("outT", [HPC * 65, seqc], F32,
                          kind="ExternalOutput").ap()

    with tile.TileContext(nc) as tc:
        with tc.tile_pool(name="const", bufs=1) as cpool, \
             tc.tile_pool(name="big", bufs=1) as bigpool, \
             tc.tile_pool(name="psP", bufs=2, space="PSUM") as psP, \
             tc.tile_pool(name="psS", bufs=2, space="PSUM") as psS, \
             tc.tile_pool(name="psAv", bufs=2, space="PSUM") as psAv, \
             tc.tile_pool(name="att", bufs=4) as att_pool, \
             tc.tile_pool(name="outp", bufs=4) as out_pool:

            # ---------------- constants ----------------
            bq_sb = cpool.tile([128, 4], F32, tag="bq")
            nc.sync.dma_start(bq_sb[:], bq.rearrange("(b p) -> p b", p=128))
            bk_sb = cpool.tile([128, 4], F32, tag="bk")
            nc.sync.dma_start(bk_sb[:], bk.rearrange("(b p) -> p b", p=128))
            bv_row = cpool.tile([1, OC], F32, tag="bv_row")
            nc.sync.dma_start(bv_row[:], bv.rearrange("(a c) -> a c", a=1))
            bv_tile = cpool.tile([128, OC], F32, tag="bv_tile")
            nc.gpsimd.partition_broadcast(bv_tile[:], bv_row[:])

            # tri[k, q] = 1 where k <= q else 0 (diagonal 128x128 block)
            tri = cpool.tile([128, 128], BF16, tag="tri")
            nc.gpsimd.memset(tri[:], 1.0)
            nc.gpsimd.affine_select(
                out=tri[:], in_=tri[:], compare_op=mybir.AluOpType.is_ge,
                fill=0.0, base=0, pattern=[[1, 128]], channel_multiplier=-1)

            # ---------------- persistent SBUF ----------------
            x_sb = bigpool.tile([128, EB * seqc], BF16, tag="x_sb")
            wq_sb = bigpool.tile([128, EB * OC], BF16, tag="wq_sb")
            wk_sb = bigpool.tile([128, EB * OC], BF16, tag="wk_sb")
            wv_sb = bigpool.tile([128, EB * OC], BF16, tag="wv_sb")
            qT_sb = bigpool.tile([128, 4 * seqc], BF16, tag="qT")
            kT_sb = bigpool.tile([128, 4 * seqc], BF16, tag="kT")
            v_aug = bigpool.tile([128, NB * HPC * 65], BF16, tag="v_aug")
            v_r = v_aug[:].rearrange("p (b h c) -> p b h c", b=NB, h=HPC)

            # parallel DMA queues: x on sync, wk on scalar, wq/wv on gpsimd
            for eb in range(EB):
                nc.scalar.dma_start(wk_sb[:, eb * OC:(eb + 1) * OC],
                                    wkT[eb * 128:(eb + 1) * 128, :])
                nc.sync.dma_start(x_sb[:, eb * seqc:(eb + 1) * seqc],
                                  xT[eb * 128:(eb + 1) * 128, :])
            for eb in range(EB):
                nc.scalar.dma_start(wq_sb[:, eb * OC:(eb + 1) * OC],
                                    wqT[eb * 128:(eb + 1) * 128, :])
            for eb in range(EB):
                nc.scalar.dma_start(wv_sb[:, eb * OC:(eb + 1) * OC],
                                    wvT[eb * 128:(eb + 1) * 128, :])

            nc.gpsimd.memset(v_r[:, :, :, 64], 1.0)

            # ---------------- projection chain groups ----------------
            # chain spec: ("k"|"q", ob, ch) weight-stationary, or ("v", sb)
            def emit_group(chains):
                pss = []
                for idx, c in enumerate(chains):
                    ps = psP.tile([128, 512], F32, tag="ps_proj",
                                  name=f"pp{idx}")
                    pss.append(ps)
                for eb in range(EB):
                    for c, ps in zip(chains, pss):
                        if c[0] == "v":
                            sb = c[1]
                            nc.tensor.matmul(
                                ps[:],
                                x_sb[:, eb * seqc + sb * 128:
                                     eb * seqc + (sb + 1) * 128],
                                wv_sb[:, eb * OC:(eb + 1) * OC],
                                start=(eb == 0), stop=(eb == EB - 1))
                        else:
                            _, ob, ch = c
                            w_sb = wk_sb if c[0] == "k" else wq_sb
                            nc.tensor.matmul(
                                ps[:, 0:CH],
                                w_sb[:, eb * OC + ob * 128:
                                     eb * OC + (ob + 1) * 128],
                                x_sb[:, eb * seqc + ch * CH:
                                     eb * seqc + (ch + 1) * CH],
                                start=(eb == 0), stop=(eb == EB - 1))
                for c, ps in zip(chains, pss):
                    if c[0] == "v":
                        sb = c[1]
                        nc.vector.tensor_add(
                            v_r[:, sb, :, 0:64],
                            ps[:].rearrange("p (h c) -> p h c", h=HPC),
                            bv_tile[:].rearrange("p (h c) -> p h c", h=HPC))
                    else:
                        _, ob, ch = c
                        dst = kT_sb if c[0] == "k" else qT_sb
                        bias_sb = bk_sb if c[0] == "k" else bq_sb
                        nc.vector.tensor_scalar_add(
                            dst[:, ob * seqc + ch * CH:
                                ob * seqc + (ch + 1) * CH],
                            ps[:, 0:CH], bias_sb[:, ob:ob + 1])

            # ---------------- attention emitters ----------------
            def widths(scn):
                """[(kb, off, w, lstart)] for chunk scn, packed offsets."""
                q0 = scn * CH
                out, off = [], 0
                for kb in range(3 * scn + 3):
                    lstart = max(0, kb * 128 - q0)
                    w = CH - lstart
                    out.append((kb, off, w, lstart))
                    off += w
                return out

            state = {}

            def emit_score_kb(p, att, sw, item):
                """Paired score MMs (head pair -> two banks of one PSUM
                tile), then ONE fused strided exp PSUM->SBUF (no DVE cast),
                then gpsimd tri-mask on the diagonal block."""
                scn, hp = p
                q0 = scn * CH
                kb, off, w, lstart = item
                ssb = psS.tile([128, 1024], F32, tag="ps_s")
                for i in range(2):
                    h = 2 * hp + i
                    ob, po = h // 2, (h % 2) * 64
                    nc.tensor.matmul(
                        ssb[:, i * 512:i * 512 + w],
                        kT_sb[po:po + 64,
                              ob * seqc + kb * 128:ob * seqc + (kb + 1) * 128],
                        qT_sb[po:po + 64,
                              ob * seqc + q0 + lstart:ob * seqc + q0 + CH],
                        start=True, stop=True)
                src = ssb[:].rearrange("p (i c) -> p i c", i=2)[:, :, 0:w]
                dst = att[:].rearrange("p (i c) -> p i c", i=2)[:, :, off:off + w]
                nc.scalar.activation(dst, src, AF.Exp, scale=0.125)
                if kb >= 3 * scn:  # diagonal block: causal tri mask
                    for i in range(2):
                        nc.gpsimd.tensor_mul(
                            att[:, i * sw + off:i * sw + off + 128],
                            att[:, i * sw + off:i * sw + off + 128],
                            tri[:])

            def emit_av_kb(p, att, sw, avs, item, nkb):
                scn, hp = p
                kb, off, w, lstart = item
                for i in range(2):
                    h = 2 * hp + i
                    nc.tensor.matmul(
                        avs[i][:, lstart:CH],
                        v_r[:, kb, h, :],
                        att[:, i * sw + off:i * sw + off + w],
                        start=(kb == 0), stop=(kb == nkb - 1))

            def emit_scores_plain(p):
                wl = widths(p[0])
                sw = sum(w for _, _, w, _ in wl)
                att = att_pool.tile([128, 2 * sw], BF16, tag="att")
                for item in wl:
                    emit_score_kb(p, att, sw, item)
                state[("att", p)] = (att, sw)

            def emit_av_scores(p_av, p_sco):
                """av MMs of p_av interleaved (PE-queue) with score MMs of
                p_sco so exp-paced score stalls are absorbed by av work."""
                av_items = widths(p_av[0]) if p_av else []
                nkb = len(av_items)
                if p_av:
                    att, sw_a = state.pop(("att", p_av))
                    avs = [psAv.tile([65, 512], F32, tag="ps_av",
                                     name=f"av{i}") for i in range(2)]
                    state[("avs", p_av)] = avs
                sco_items = widths(p_sco[0]) if p_sco else []
                if p_sco:
                    sw_s = sum(w for _, _, w, _ in sco_items)
                    att_s = att_pool.tile([128, 2 * sw_s], BF16, tag="att")
                    state[("att", p_sco)] = (att_s, sw_s)
                for t in range(max(len(av_items), len(sco_items))):
                    if t < len(sco_items):
                        emit_score_kb(p_sco, att_s, sw_s, sco_items[t])
                    if t < len(av_items):
                        emit_av_kb(p_av, att, sw_a, avs, av_items[t], nkb)

            def emit_out(p):
                scn, hp = p
                q0 = scn * CH
                avs = state.pop(("avs", p))
                for i in range(2):
                    h = 2 * hp + i
                    o_sb = out_pool.tile([65, CH], F32, tag="osb",
                                         name="o_sb")
                    nc.vector.tensor_copy(o_sb[:], avs[i][:, 0:CH])
                    nc.sync.dma_start(
                        outT[h * 65:(h + 1) * 65, q0:q0 + CH], o_sb[:])

            # ---------------- schedule ----------------
            # Upfront eb-major groups overlap the x DMA; k/q chunk 0 and
            # v blocks 0-2 must precede the first attention slot.
            emit_group([("k", 0, 0), ("k", 1, 0)])
            emit_group([("k", 2, 0), ("k", 3, 0)])
            emit_group([("q", 0, 0), ("q", 1, 0)])
            emit_group([("q", 2, 0), ("q", 3, 0)])
            emit_group([("v", 0), ("v", 1)])
            emit_group([("v", 2)])

            # remaining chains spread over attention slots (need-by safe:
            # S(c,*) needs kT chunks<=c + qT chunk c two slots early; av(c,*)
            # needs v blocks < 3c+3 by its own slot)
            def G(*chains):
                return lambda: emit_group(list(chains))
            sched = {}
            if NCH == 3:
                sched = {
                    (0, 0): [G(("k", 0, 1), ("k", 1, 1))],
                    (0, 1): [G(("k", 2, 1), ("k", 3, 1)), G(("v", 3))],
                    (0, 2): [G(("q", 0, 1), ("q", 1, 1)), G(("v", 4))],
                    (0, 3): [G(("q", 2, 1), ("q", 3, 1)), G(("v", 5))],
                    (1, 0): [G(("k", 0, 2), ("k", 1, 2))],
                    (1, 1): [G(("k", 2, 2), ("k", 3, 2)), G(("v", 6))],
                    (1, 2): [G(("q", 0, 2), ("q", 1, 2)), G(("v", 7))],
                    (1, 3): [G(("q", 2, 2), ("q", 3, 2))],
                    (2, 0): [G(("v", 8))],
                }
            else:
                for ch in range(1, NCH):
                    for ob in range(0, 4, 2):
                        emit_group([("k", ob, ch), ("k", ob + 1, ch)])
                        emit_group([("q", ob, ch), ("q", ob + 1, ch)])
                for sb in range(3, NB):
                    emit_group([("v", sb)])

            pairs = [(scn, hp) for scn in range(NCH) for hp in range(4)]
            n = len(pairs)
            emit_scores_plain(pairs[0])
            emit_scores_plain(pairs[1])
            for j, p in enumerate(pairs):
                for fn in sched.get(p, ()):
                    fn()
                emit_av_scores(p, pairs[j + 2] if j + 2 < n else None)
                emit_out(p)

    nc.compile()
    return nc


def get_nc(seqc=CAP0):
    if seqc not in _cache:
        _cache[seqc] = _build_nc(seqc)
    return _cache[seqc]


def _prep(input_x, pad_mask, Wq, bq, Wk, bk, Wv, bv):
    import ml_dtypes
    bf16 = ml_dtypes.bfloat16
    input_x = np.asarray(input_x, dtype=np.float32)
    pad = np.asarray(pad_mask)
    Ws = [np.asarray(w, dtype=np.float32) for w in (Wq, Wk, Wv)]
    bs = [np.ascontiguousarray(np.asarray(v, dtype=np.float32))
          for v in (bq, bk, bv)]

    idxs = [np.flatnonzero(pad[b]) for b in range(B)]
    sbs = [len(ix) for ix in idxs]
    cap = max(CAP0, -(-max(sbs) // CH) * CH)

    xTs = []
    for b in range(B):
        xc = np.zeros((cap, E), np.float32)
        xc[:sbs[b]] = input_x[b][idxs[b]]
        xTs.append(np.ascontiguousarray(xc.T).astype(bf16))

    wslices = {}
    for g in range(2):
        sl = slice(g * OC, (g + 1) * OC)
        wslices[g] = tuple(
            np.ascontiguousarray(W[sl].T).astype(bf16) for W in Ws
        ) + tuple(np.ascontiguousarray(v[sl]) for v in bs)

    in_maps = []
    for c in range(NCORES):
        b, g = c // 2, c % 2
        wq_t, wk_t, wv_t, bq_s, bk_s, bv_s = wslices[g]
        in_maps.append({
            "xT": xTs[b], "wqT": wq_t, "wkT": wk_t, "wvT": wv_t,
            "bq": bq_s, "bk": bk_s, "bv": bv_s,
        })
    return in_maps, idxs, sbs, cap


def _assemble(results, idxs, sbs, S):
    out = np.zeros((B, S, E), dtype=np.float32)
    for c in range(NCORES):
        b, g = c // 2, c % 2
        arr = results[c]["outT"]  # [520, cap] f32
        nb = sbs[b]
        for h in range(HPC):
            blk = arr[h * 65:(h + 1) * 65, :nb]
            o = blk[:64] / blk[64:65]
            out[b, idxs[b], g * OC + h * 64:g * OC + (h + 1) * 64] = o.T
    return out


LAST_RESULT = None


def kernel(input_x, pad_mask, Wq, bq, Wk, bk, Wv, bv):
    from concourse.bass_utils import run_bass_kernel_spmd
    global LAST_RESULT
    S = np.asarray(input_x).shape[1]
    in_maps, idxs, sbs, cap = _prep(input_x, pad_mask, Wq, bq, Wk, bk, Wv, bv)
    nc = get_nc(cap)
    res = run_bass_kernel_spmd(nc, in_maps, core_ids=list(range(NCORES)))
    LAST_RESULT = res
    if res.exec_time_ns is not None:
        print(f"HW exec time: {res.exec_time_ns} ns")
    return _assemble(res.results, idxs, sbs, S)


# revision 13
# speedup vs baseline: 4.3238x; 1.0671x over previous
"""Multi-head causal+padded attention on 8 Trainium2 NeuronCores.

Core c handles batch b = c//2 and head-group g = c%2 (8 of 16 heads).

Pad compaction: the reference masks out padded keys/queries entirely
(padded query rows output 0). Attention over the pad-compacted sequence is
exactly equivalent, so the host gathers the ~1024 unpadded rows per batch,
zero-pads to a fixed 1152 capacity, and the device runs a causal MHA on
[1152]. Outputs are scattered back with zeros in padded rows.

Device (per core, all-bf16 datapath, fp32 PSUM):
  qT/kT = W^T-slices @ xT in [out, seq] layout; v in natural [seq, out]
  layout augmented with a ones column (softmax denominator rides along the
  att@v accumulation chain). Scores transposed per 128-k-block, head pairs
  packed into PE row-groups 0-1/2-3 (concurrent matmuls), DVE-copied
  PSUM->SBUF, one batched exp per head-pair-chunk on the scalar engine,
  tri-masked on GpSimd, then att^T-chained into [65, 384] PSUM; ScalarE
  evacuates unnormalized out+denominator; the host divides.

  Emission is software-pipelined with stage offsets so no engine queue
  blocks another: slot j emits [proj chains | exp+mask(j+1) | scores(j+2)
  interleaved with av(j) | out(j)]. Projection chains are spread across
  attention slots (eb-major groups at startup to overlap the x DMA) so the
  PE never idles long enough for HAM to re-throttle the clock.
"""
import os
import sys

sys.path.insert(0, "/opt/trn_rl_repo")

import numpy as np

E = 1024
D = 64
HPC = 8         # heads per core
OC = HPC * D    # 512 output dims per core
EB = E // 128   # 8 contraction blocks
B = 4
NCORES = 8
CH = 384        # q-chunk width
CAP0 = 1152     # default compacted seq capacity (multiple of 384)

_cache = {}


def _build_nc(seqc):
    from concourse import bacc
    import concourse.tile as tile
    import concourse.mybir as mybir

    assert seqc % CH == 0
    NCH = seqc // CH          # q-chunks (3 at cap 1152)
    NB = seqc // 128          # 128-k-blocks (9)
    F32 = mybir.dt.float32
    BF16 = mybir.dt.bfloat16
    AF = mybir.ActivationFunctionType

    nc = bacc.Bacc("TRN2", target_bir_lowering=False, debug=False,
                   num_devices=NCORES)
    xT = nc.dram_tensor("xT", [E, seqc], BF16, kind="ExternalInput").ap()
    wqT = nc.dram_tensor("wqT", [E, OC], BF16, kind="ExternalInput").ap()
    wkT = nc.dram_tensor("wkT", [E, OC], BF16, kind="ExternalInput").ap()
    wvT = nc.dram_tensor("wvT", [E, OC], BF16, kind="ExternalInput").ap()
    bq = nc.dram_tensor("bq", [OC], F32, kind="ExternalInput").ap()
    bk = nc.dram_tensor("bk", [OC], F32, kind="ExternalInput").ap()
    bv = nc.dram_tensor("bv", [OC], F32, kind="ExternalInput").ap()
    # unnormalized out (64 rows) + denominator (row 64) per head
    outT = nc.dram_tensor("outT", [HPC * 65, seqc], F32,
                          kind="ExternalOutput").ap()

    with tile.TileContext(nc) as tc:
        with tc.tile_pool(name="const", bufs=1) as cpool, \
             tc.tile_pool(name="big", bufs=1) as bigpool, \
             tc.tile_pool(name="psP", bufs=2, space="PSUM") as psP, \
             tc.tile_pool(name="psS", bufs=2, space="PSUM") as psS, \
             tc.tile_pool(name="psAv", bufs=2, space="PSUM") as psAv, \
             tc.tile_pool(name="att", bufs=4) as att_pool, \
             tc.tile_pool(name="outp", bufs=4) as out_pool:

            # ---------------- constants ----------------
            bq_sb = cpool.tile([128, 4], F32, tag="bq")
            nc.sync.dma_start(bq_sb[:], bq.rearrange("(b p) -> p b", p=128))
            # hoist the exp ACT_TABLE_LOAD (~2.7us) into the DMA window
            warm = cpool.tile([1, 4], F32, tag="warm")
            nc.scalar.activation(warm[:], bq_sb[0:1, :], AF.Exp, scale=0.0)
            bk_sb = cpool.tile([128, 4], F32, tag="bk")
            nc.sync.dma_start(bk_sb[:], bk.rearrange("(b p) -> p b", p=128))
            bv_row = cpool.tile([1, OC], F32, tag="bv_row")
            nc.sync.dma_start(bv_row[:], bv.rearrange("(a c) -> a c", a=1))
            bv_tile = cpool.tile([128, OC], F32, tag="bv_tile")
            nc.gpsimd.partition_broadcast(bv_tile[:], bv_row[:])

            # tri[k, q] = 1 where k <= q else 0 (diagonal 128x128 block)
            tri = cpool.tile([128, 128], BF16, tag="tri")
            nc.gpsimd.memset(tri[:], 1.0)
            nc.gpsimd.affine_select(
                out=tri[:], in_=tri[:], compare_op=mybir.AluOpType.is_ge,
                fill=0.0, base=0, pattern=[[1, 128]], channel_multiplier=-1)

            # ---------------- persistent SBUF ----------------
            x_sb = bigpool.tile([128, EB * seqc], BF16, tag="x_sb")
            wq_sb = bigpool.tile([128, EB * OC], BF16, tag="wq_sb")
            wk_sb = bigpool.tile([128, EB * OC], BF16, tag="wk_sb")
            wv_sb = bigpool.tile([128, EB * OC], BF16, tag="wv_sb")
            qT_sb = bigpool.tile([128, 4 * seqc], BF16, tag="qT")
            kT_sb = bigpool.tile([128, 4 * seqc], BF16, tag="kT")
            v_aug = bigpool.tile([128, NB * HPC * 65], BF16, tag="v_aug")
            v_r = v_aug[:].rearrange("p (b h c) -> p b h c", b=NB, h=HPC)

            # parallel DMA queues: x on sync, wk on scalar, wq/wv on gpsimd
            for eb in range(EB):
                nc.scalar.dma_start(wk_sb[:, eb * OC:(eb + 1) * OC],
                                    wkT[eb * 128:(eb + 1) * 128, :])
                nc.sync.dma_start(x_sb[:, eb * seqc:(eb + 1) * seqc],
                                  xT[eb * 128:(eb + 1) * 128, :])
            for eb in range(EB):
                nc.scalar.dma_start(wq_sb[:, eb * OC:(eb + 1) * OC],
                                    wqT[eb * 128:(eb + 1) * 128, :])
            for eb in range(EB):
                nc.scalar.dma_start(wv_sb[:, eb * OC:(eb + 1) * OC],
                                    wvT[eb * 128:(eb + 1) * 128, :])

            nc.gpsimd.memset(v_r[:, :, :, 64], 1.0)

            # ---------------- projection chain groups ----------------
            # chain spec: ("k"|"q", ob, ch) weight-stationary, or ("v", sb)
            def emit_group(chains):
                pss = []
                for idx, c in enumerate(chains):
                    if idx < 2:
                        ps = psP.tile([128, 512], F32, tag="ps_proj",
                                      name=f"pp{idx}")
                    else:
                        # loan a psS pair-tile (2 banks) for startup groups
                        # of 4 chains; attention hasn't started yet
                        if idx == 2:
                            loan = psS.tile([128, 1024], F32, tag="ps_s",
                                            name="loan")
                        ps = loan[:, (idx - 2) * 512:(idx - 1) * 512]
                    pss.append(ps)
                for eb in range(EB):
                    for c, ps in zip(chains, pss):
                        if c[0] == "v":
                            sb = c[1]
                            nc.tensor.matmul(
                                ps[:],
                                x_sb[:, eb * seqc + sb * 128:
                                     eb * seqc + (sb + 1) * 128],
                                wv_sb[:, eb * OC:(eb + 1) * OC],
                                start=(eb == 0), stop=(eb == EB - 1))
                        else:
                            _, ob, ch = c
                            w_sb = wk_sb if c[0] == "k" else wq_sb
                            nc.tensor.matmul(
                                ps[:, 0:CH],
                                w_sb[:, eb * OC + ob * 128:
                                     eb * OC + (ob + 1) * 128],
                                x_sb[:, eb * seqc + ch * CH:
                                     eb * seqc + (ch + 1) * CH],
                                start=(eb == 0), stop=(eb == EB - 1))
                for c, ps in zip(chains, pss):
                    if c[0] == "v":
                        sb = c[1]
                        nc.vector.tensor_add(
                            v_r[:, sb, :, 0:64],
                            ps[:].rearrange("p (h c) -> p h c", h=HPC),
                            bv_tile[:].rearrange("p (h c) -> p h c", h=HPC))
                    else:
                        _, ob, ch = c
                        dst = kT_sb if c[0] == "k" else qT_sb
                        bias_sb = bk_sb if c[0] == "k" else bq_sb
                        nc.vector.tensor_scalar_add(
                            dst[:, ob * seqc + ch * CH:
                                ob * seqc + (ch + 1) * CH],
                            ps[:, 0:CH], bias_sb[:, ob:ob + 1])

            # ---------------- attention emitters ----------------
            def widths(scn):
                """[(kb, off, w, lstart)] for chunk scn, packed offsets."""
                q0 = scn * CH
                out, off = [], 0
                for kb in range(3 * scn + 3):
                    lstart = max(0, kb * 128 - q0)
                    w = CH - lstart
                    out.append((kb, off, w, lstart))
                    off += w
                return out

            state = {}

            def emit_score_kb(p, att, sw, item):
                """Paired score MMs (head pair -> two banks of one PSUM
                tile), then ONE fused strided exp PSUM->SBUF (no DVE cast),
                then gpsimd tri-mask on the diagonal block."""
                scn, hp = p
                q0 = scn * CH
                kb, off, w, lstart = item
                ssb = psS.tile([128, 1024], F32, tag="ps_s")
                for i in range(2):
                    h = 2 * hp + i
                    ob, po = h // 2, (h % 2) * 64
                    nc.tensor.matmul(
                        ssb[:, i * 512:i * 512 + w],
                        kT_sb[po:po + 64,
                              ob * seqc + kb * 128:ob * seqc + (kb + 1) * 128],
                        qT_sb[po:po + 64,
                              ob * seqc + q0 + lstart:ob * seqc + q0 + CH],
                        start=True, stop=True)
                src = ssb[:].rearrange("p (i c) -> p i c", i=2)[:, :, 0:w]
                dst = att[:].rearrange("p (i c) -> p i c", i=2)[:, :, off:off + w]
                nc.scalar.activation(dst, src, AF.Exp, scale=0.125)
                if kb >= 3 * scn:  # diagonal block: causal tri mask
                    for i in range(2):
                        nc.gpsimd.tensor_mul(
                            att[:, i * sw + off:i * sw + off + 128],
                            att[:, i * sw + off:i * sw + off + 128],
                            tri[:])

            def emit_av_kb(p, att, sw, avs, item, nkb):
                scn, hp = p
                kb, off, w, lstart = item
                for i in range(2):
                    h = 2 * hp + i
                    nc.tensor.matmul(
                        avs[i][:, lstart:CH],
                        v_r[:, kb, h, :],
                        att[:, i * sw + off:i * sw + off + w],
                        start=(kb == 0), stop=(kb == nkb - 1))

            def emit_scores_plain(p):
                wl = widths(p[0])
                sw = sum(w for _, _, w, _ in wl)
                att = att_pool.tile([128, 2 * sw], BF16, tag="att")
                for item in wl:
                    emit_score_kb(p, att, sw, item)
                state[("att", p)] = (att, sw)

            def emit_av_scores(p_av, p_sco):
                """av MMs of p_av interleaved (PE-queue) with score MMs of
                p_sco so exp-paced score stalls are absorbed by av work."""
                av_items = widths(p_av[0]) if p_av else []
                nkb = len(av_items)
                if p_av:
                    att, sw_a = state.pop(("att", p_av))
                    avs = [psAv.tile([65, 512], F32, tag="ps_av",
                                     name=f"av{i}") for i in range(2)]
                    state[("avs", p_av)] = avs
                sco_items = widths(p_sco[0]) if p_sco else []
                if p_sco:
                    sw_s = sum(w for _, _, w, _ in sco_items)
                    att_s = att_pool.tile([128, 2 * sw_s], BF16, tag="att")
                    state[("att", p_sco)] = (att_s, sw_s)
                for t in range(max(len(av_items), len(sco_items))):
                    if t < len(sco_items):
                        emit_score_kb(p_sco, att_s, sw_s, sco_items[t])
                    if t < len(av_items):
                        emit_av_kb(p_av, att, sw_a, avs, av_items[t], nkb)

            def emit_out(p):
                scn, hp = p
                q0 = scn * CH
                avs = state.pop(("avs", p))
                for i in range(2):
                    h = 2 * hp + i
                    o_sb = out_pool.tile([65, CH], F32, tag="osb",
                                         name="o_sb")
                    nc.vector.tensor_copy(o_sb[:], avs[i][:, 0:CH])
                    nc.sync.dma_start(
                        outT[h * 65:(h + 1) * 65, q0:q0 + CH], o_sb[:])

            # ---------------- schedule ----------------
            # Upfront eb-major groups overlap the x DMA; k/q chunk 0 and
            # v blocks 0-2 must precede the first attention slot.
            emit_group([("k", 0, 0), ("k", 1, 0), ("k", 2, 0), ("k", 3, 0)])
            emit_group([("q", 0, 0), ("q", 1, 0), ("q", 2, 0), ("q", 3, 0)])

            # remaining chains spread over attention slots (need-by safe:
            # S(c,*) needs kT chunks<=c + qT chunk c two slots early; av(c,*)
            # needs v blocks < 3c+3 by its own slot)
            def G(*chains):
                return lambda: emit_group(list(chains))
            sched = {}
            if NCH == 3:
                sched = {
                    (0, 0): [G(("k", 0, 1), ("k", 1, 1))],
                    (0, 1): [G(("k", 2, 1), ("k", 3, 1)), G(("v", 3))],
                    (0, 2): [G(("q", 0, 1), ("q", 1, 1)), G(("v", 4))],
                    (0, 3): [G(("q", 2, 1), ("q", 3, 1)), G(("v", 5))],
                    (1, 0): [G(("k", 0, 2), ("k", 1, 2))],
                    (1, 1): [G(("k", 2, 2), ("k", 3, 2)), G(("v", 6))],
                    (1, 2): [G(("q", 0, 2), ("q", 1, 2)), G(("v", 7))],
                    (1, 3): [G(("q", 2, 2), ("q", 3, 2))],
                    (2, 0): [G(("v", 8))],
                }
            else:
                for ch in range(1, NCH):
                    for ob in range(0, 4, 2):
                        emit_group([("k", ob, ch), ("k", ob + 1, ch)])
                        emit_group([("q", ob, ch), ("q", ob + 1, ch)])
                for sb in range(3, NB):
                    emit_group([("v", sb)])

            pairs = [(scn, hp) for scn in range(NCH) for hp in range(4)]
            n = len(pairs)
            emit_scores_plain(pairs[0])
            emit_scores_plain(pairs[1])
            # v blocks 0-2 after the first score blocks: PE starts attention
            # sooner; av(0,0) still sees them in order
            emit_group([("v", 0), ("v", 1)])
            emit_group([("v", 2)])
            for j, p in enumerate(pairs):
                for fn in sched.get(p, ()):
                    fn()
                emit_av_scores(p, pairs[j + 2] if j + 2 < n else None)
                emit_out(p)

    nc.compile()
    return nc


def get_nc(seqc=CAP0):
    if seqc not in _cache:
        _cache[seqc] = _build_nc(seqc)
    return _cache[seqc]


def _prep(input_x, pad_mask, Wq, bq, Wk, bk, Wv, bv):
    import ml_dtypes
    bf16 = ml_dtypes.bfloat16
    input_x = np.asarray(input_x, dtype=np.float32)
    pad = np.asarray(pad_mask)
    Ws = [np.asarray(w, dtype=np.float32) for w in (Wq, Wk, Wv)]
    bs = [np.ascontiguousarray(np.asarray(v, dtype=np.float32))
          for v in (bq, bk, bv)]

    idxs = [np.flatnonzero(pad[b]) for b in range(B)]
    sbs = [len(ix) for ix in idxs]
    cap = max(CAP0, -(-max(sbs) // CH) * CH)

    xTs = []
    for b in range(B):
        xc = np.zeros((cap, E), np.float32)
        xc[:sbs[b]] = input_x[b][idxs[b]]
        xTs.append(np.ascontiguousarray(xc.T).astype(bf16))

    wslices = {}
    for g in range(2):
        sl = slice(g * OC, (g + 1) * OC)
        wslices[g] = tuple(
            np.ascontiguousarray(W[sl].T).astype(bf16) for W in Ws
        ) + tuple(np.ascontiguousarray(v[sl]) for v in bs)

    in_maps = []
    for c in range(NCORES):
        b, g = c // 2, c % 2
        wq_t, wk_t, wv_t, bq_s, bk_s, bv_s = wslices[g]
        in_maps.append({
            "xT": xTs[b], "wqT": wq_t, "wkT": wk_t, "wvT": wv_t,
            "bq": bq_s, "bk": bk_s, "bv": bv_s,
        })
    return in_maps, idxs, sbs, cap


def _assemble(results, idxs, sbs, S):
    out = np.zeros((B, S, E), dtype=np.float32)
    for c in range(NCORES):
        b, g = c // 2, c % 2
        arr = results[c]["outT"]  # [520, cap] f32
        nb = sbs[b]
        for h in range(HPC):
            blk = arr[h * 65:(h + 1) * 65, :nb]
            o = blk[:64] / blk[64:65]
            out[b, idxs[b], g * OC + h * 64:g * OC + (h + 1) * 64] = o.T
    return out


LAST_RESULT = None


def kernel(input_x, pad_mask, Wq, bq, Wk, bk, Wv, bv):
    from concourse.bass_utils import run_bass_kernel_spmd
    global LAST_RESULT
    S = np.asarray(input_x).shape[1]
    in_maps, idxs, sbs, cap = _prep(input_x, pad_mask, Wq, bq, Wk, bk, Wv, bv)
    nc = get_nc(cap)
    res = run_bass_kernel_spmd(nc, in_maps, core_ids=list(range(NCORES)))
    LAST_RESULT = res
    if res.exec_time_ns is not None:
        print(f"HW exec time: {res.exec_time_ns} ns")
    return _assemble(res.results, idxs, sbs, S)
